# revision 1
# baseline (speedup 1.0000x reference)
"""Trainium2 Bass kernel for Mllama-style GQA self-attention (B=1, S=2048,
H=32 q-heads, KVH=8 kv-heads, D=128), tensor-parallel over heads across 8
NeuronCores.

Sharding: core c owns kv-head c and q-heads 4c..4c+3 (Wq/Wk/Wv column shards),
computes its heads' attention output in transposed [feature, seq] layout,
AllGathers the 4096-feature activation across cores, then computes a
512-column shard of the output projection (Wo row shard). Host concatenates
the 8 output shards along the feature axis.

All matmuls run in bf16 (fp32 PSUM accumulation). Softmax skips the max
subtraction (scores are O(10) here so exp is safe in fp32) and normalizes
after the probs @ V matmul via a ones-row K=1 broadcast matmul; that
normalize chain is software-pipelined one attention unit behind the matmul
stream so the PE never waits on it.
"""
import math
from contextlib import ExitStack
import numpy as np
import ml_dtypes

import concourse.bass as bass
import concourse.bacc as bacc
import concourse.mybir as mybir
import concourse.tile as tile
from concourse.bass_utils import run_bass_kernel_spmd

BF16 = ml_dtypes.bfloat16
S, E, H, KVH, D = 2048, 4096, 32, 8, 128
N_CORES = 8
G = H // KVH                      # q heads per core
OSH = G * D                       # per-core q/attn feature count (512)
PC = 512                          # phase-1 seq chunk (projection rhs width)
AC = 512                          # attention sq chunk width
N_PC = S // PC                    # 8
N_AC = S // AC                    # 4
NE = E // 128                     # 32 contraction tiles
N_ST = S // 128                   # 16 seq tiles

_BUILD_CACHE = {}


def build_bass(causal: bool):
    key = causal
    if key in _BUILD_CACHE:
        return _BUILD_CACHE[key]
    dt = mybir.dt
    nc = bacc.Bacc("TRN2", target_bir_lowering=False, debug=False,
                   enable_asserts=False, num_devices=N_CORES)

    XT4 = nc.dram_tensor("xt", [N_PC, 128, NE, PC], dt.bfloat16, kind="ExternalInput").ap()
    WQT = nc.dram_tensor("wqt", [128, NE, OSH], dt.bfloat16, kind="ExternalInput").ap()
    WKT = nc.dram_tensor("wkt", [128, NE, D], dt.bfloat16, kind="ExternalInput").ap()
    WVT = nc.dram_tensor("wvt", [128, NE, D], dt.bfloat16, kind="ExternalInput").ap()
    WOT = nc.dram_tensor("wot", [128, G, E], dt.bfloat16, kind="ExternalInput").ap()
    ROPE = nc.dram_tensor("rope", [4, D, S], dt.bfloat16, kind="ExternalInput").ap()
    TRI = nc.dram_tensor("tri", [4, 128, AC], dt.bfloat16, kind="ExternalInput").ap()
    OUT = nc.dram_tensor("out", [S, E], dt.float32, kind="ExternalOutput").ap()

    with tile.TileContext(nc) as tc:
        with (
            tc.tile_pool(name="wpool", bufs=1) as wpool,
            tc.tile_pool(name="qkv", bufs=1) as qkvpool,
            tc.tile_pool(name="consts", bufs=1) as cpool,
            tc.tile_pool(name="epool", bufs=3) as epool,
            tc.tile_pool(name="small", bufs=2) as smallpool,
            tc.tile_pool(name="attn", bufs=9) as attnpool,
            tc.tile_pool(name="outs", bufs=2) as outpool,
            tc.tile_pool(name="ps_qkv", bufs=2, space="PSUM") as ps_qkv,
            tc.tile_pool(name="ps_s", bufs=3, space="PSUM") as ps_s,
            tc.tile_pool(name="ps_ot", bufs=3, space="PSUM") as ps_ot,
        ):
            # phase-1-only pools; closed after phase 1 so the o_proj input
            # pool (agp) can reuse their SBUF space
            p1ctx = ExitStack()
            xspool = p1ctx.enter_context(tc.tile_pool(name="xs", bufs=2))
            cspool = p1ctx.enter_context(tc.tile_pool(name="cs", bufs=2))
            rtmppool = p1ctx.enter_context(tc.tile_pool(name="rtmp", bufs=1))

            # --- resident weights, [128, NE, width]. Priority order: the
            # first chunk's activations and Wk jump ahead of the big weight
            # burst so the PE starts in ~10us instead of ~70.
            wq_sb = wpool.tile([128, NE, OSH], dt.bfloat16)
            wk_sb = wpool.tile([128, NE, D], dt.bfloat16)
            wv_sb = wpool.tile([128, NE, D], dt.bfloat16)
            wo_sb = wpool.tile([128, G, E], dt.bfloat16)

            xs0 = xspool.tile([128, NE, PC], dt.bfloat16, tag="xs")
            cs0 = cspool.tile([128, 4, PC], dt.bfloat16, tag="cs")
            for q in range(8):
                nc.sync.dma_start(xs0[:, q * 4:(q + 1) * 4, :],
                                  XT4[0, :, q * 4:(q + 1) * 4, :])
            for q in range(8):
                nc.sync.dma_start(wk_sb[:, q * 4:(q + 1) * 4, :],
                                  WKT[:, q * 4:(q + 1) * 4, :])
            for q in range(8):
                nc.sync.dma_start(wv_sb[:, q * 4:(q + 1) * 4, :],
                                  WVT[:, q * 4:(q + 1) * 4, :])
            nc.sync.dma_start(cs0[:], ROPE[:, :, 0:PC].rearrange("j p s -> p j s"))
            for q in range(8):
                nc.sync.dma_start(wq_sb[:, q * 4:(q + 1) * 4, :],
                                  WQT[:, q * 4:(q + 1) * 4, :])

            tri_sb = cpool.tile([128, 4, AC], dt.bfloat16)
            nc.sync.dma_start(tri_sb[:], TRI.rearrange("j p f -> p j f"))
            ones_col = cpool.tile([128, 1], dt.bfloat16)
            nc.vector.memset(ones_col[:], 1.0)
            ones_row = cpool.tile([1, 128], dt.bfloat16)
            nc.vector.memset(ones_row[:], 1.0)

            # --- persistent activations
            qT_sb = qkvpool.tile([128, G, S], dt.bfloat16)     # per-head [d, s]
            kT_sb = qkvpool.tile([128, S], dt.bfloat16)        # [d, s]
            v_sb = qkvpool.tile([128, N_ST, D], dt.bfloat16)   # per s-tile [t, d]

            # ================= Phase 1: QKV projections + RoPE ==============
            for sc in range(N_PC):
                s0 = sc * PC
                if sc == 0:
                    xs, cs = xs0, cs0
                else:
                    xs = xspool.tile([128, NE, PC], dt.bfloat16, tag="xs")
                    for q in range(4):
                        nc.sync.dma_start(xs[:, q * 8:(q + 1) * 8, :],
                                          XT4[sc, :, q * 8:(q + 1) * 8, :])
                    cs = cspool.tile([128, 4, PC], dt.bfloat16, tag="cs")
                    nc.sync.dma_start(cs[:], ROPE[:, :, s0:s0 + PC]
                                      .rearrange("j p s -> p j s"))
                cosq_t, sinq_t = cs[:, 0, :], cs[:, 1, :]
                cosk_t, sink_t = cs[:, 2, :], cs[:, 3, :]

                # k head first, then v, then q heads (k/v weights land first)
                for hh in [G, -1] + list(range(G)):
                    if hh == -1:
                        for u in range(PC // 128):
                            st = (s0 // 128) + u
                            pv = ps_qkv.tile([128, D], dt.float32, tag="pq")
                            for e in range(NE):
                                nc.tensor.matmul(pv[:],
                                                 xs[:, e, u * 128:(u + 1) * 128],
                                                 wv_sb[:, e, :],
                                                 start=(e == 0), stop=(e == NE - 1))
                            nc.vector.tensor_copy(v_sb[:, st, :], pv[:])
                        continue
                    is_k = hh == G
                    pq = ps_qkv.tile([128, PC], dt.float32, tag="pq")
                    for e in range(NE):
                        lhsT = (wk_sb[:, e, :] if is_k
                                else wq_sb[:, e, hh * D:(hh + 1) * D])
                        nc.tensor.matmul(pq[:], lhsT, xs[:, e, :],
                                         start=(e == 0), stop=(e == NE - 1))
                    cos_t, sin_t = (cosk_t, sink_t) if is_k else (cosq_t, sinq_t)
                    dest = kT_sb[:, s0:s0 + PC] if is_k \
                        else qT_sb[:, hh, s0:s0 + PC]
                    rt = rtmppool.tile([128, 2, PC], dt.float32, tag="rt")
                    t1, t2 = rt[:, 0, :], rt[:, 1, :]
                    # low half: q'= q_lo*cos_lo + q_hi*sin_mod_lo
                    nc.vector.tensor_mul(t1[0:64, :], pq[0:64, :], cos_t[0:64, :])
                    nc.vector.tensor_mul(t2[0:64, :], pq[64:128, :], sin_t[0:64, :])
                    nc.vector.tensor_add(dest[0:64, :], t1[0:64, :], t2[0:64, :])
                    # high half: q'= q_hi*cos_hi + q_lo*sin_mod_hi
                    nc.vector.tensor_mul(t1[64:128, :], pq[64:128, :], cos_t[64:128, :])
                    nc.vector.tensor_mul(t2[64:128, :], pq[0:64, :], sin_t[64:128, :])
                    nc.vector.tensor_add(dest[64:128, :], t1[64:128, :], t2[64:128, :])


            p1ctx.close()

            # Wo (own-feature rows, all 4096 cols) deferred past startup DMAs
            for q in range(4):
                nc.sync.dma_start(wo_sb[:, :, q * 1024:(q + 1) * 1024],
                                  WOT[:, :, q * 1024:(q + 1) * 1024])

            # ============ Phase 2 + partial o_proj ==========================
            # Attention as (sq-chunk, head) units with a depth-2 pipelined
            # normalize tail; chunks 0 and 1 interleaved so tiny early units
            # give the tail chain runway. As soon as a chunk's 4 heads are
            # normalized, the partial output projection for those rows runs
            # (contraction over this core's 512 features only) and the
            # full-width partial rows stream out; the host sums the 8 cores'
            # partials. No collectives; the final unit's o_proj is the only
            # serial tail.
            pending = []
            attnTs = {}             # key -> {h: attnT tile}
            kmeta = {}
            remaining = {}

            def make_units():
                units, keys = [], []
                order = [0, 1] if causal else [0]
                if causal:
                    ks = [(str(j), j * AC, AC, 4 * (j + 1), 4 * j)
                          for j in range(N_AC)]
                else:
                    ks = [(str(j), j * AC, AC, N_ST, N_ST) for j in range(N_AC)]
                keys = ks
                if causal:
                    for h in range(G):
                        for j in (0, 1):
                            units.append(ks[j] + (h,))
                    for j in (2, 3):
                        units += [ks[j] + (h,) for h in range(G)]
                else:
                    for j in range(N_AC):
                        units += [ks[j] + (h,) for h in range(G)]
                return units, keys

            units, keys = make_units()
            for k in keys:
                kmeta[k[0]] = (k[1], k[2])
                remaining[k[0]] = G
                attnTs[k[0]] = {}

            def emit_oproj(key):
                c0, cw = kmeta[key]
                ats = attnTs[key]
                for t in range(cw // 128):
                    for pc in range(8):
                        po = ps_qkv.tile([128, OSH], dt.float32, tag="pq")
                        for hh in range(G):
                            nc.tensor.matmul(
                                po[:], ats[hh][:, t * 128:(t + 1) * 128],
                                wo_sb[:, hh, pc * OSH:(pc + 1) * OSH],
                                start=(hh == 0), stop=(hh == G - 1))
                        o_sb = outpool.tile([128, OSH], dt.float32, tag="o")
                        nc.vector.tensor_copy(o_sb[:], po[:])
                        nc.sync.dma_start(
                            OUT[c0 + t * 128: c0 + (t + 1) * 128,
                                pc * OSH:(pc + 1) * OSH], o_sb[:])

            def emit_tail(u):
                key, c0, cw, nb, d0, h, ot, esum = u
                den = ps_s.tile([1, cw], dt.float32, tag="st")
                nc.tensor.matmul(den[:], ones_col[:], esum[:],
                                 start=True, stop=True)
                recip = smallpool.tile([1, cw], dt.bfloat16, tag="recip")
                with nc.allow_low_precision(reason="softmax denom recip bf16"):
                    nc.vector.reciprocal(recip[:], den[:])
                bc_ps = ps_s.tile([128, cw], dt.float32, tag="st")
                nc.tensor.matmul(bc_ps[:], ones_row[:], recip[:],
                                 start=True, stop=True)
                bc_sb = smallpool.tile([128, cw], dt.bfloat16, tag="bc")
                nc.scalar.copy(bc_sb[:], bc_ps[:])
                attnT = attnpool.tile([128, cw], dt.bfloat16, tag="attnT")
                nc.vector.tensor_mul(attnT[:], ot[:], bc_sb[:])
                attnTs[key][h] = attnT
                remaining[key] -= 1
                if remaining[key] == 0:
                    emit_oproj(key)

            for i, (key, c0, cw, nb, d0, h) in enumerate(units):
                ot = ps_ot.tile([128, cw], dt.float32, tag="ot")
                esum = epool.tile([128, cw], dt.bfloat16, tag="esum")
                for b in range(nb):
                    st_ps = ps_s.tile([128, cw], dt.float32, tag="st")
                    nc.tensor.matmul(st_ps[:], kT_sb[:, b * 128:(b + 1) * 128],
                                     qT_sb[:, h, c0:c0 + cw],
                                     start=True, stop=True)
                    e_sb = epool.tile([128, cw], dt.bfloat16, tag="e")
                    nc.scalar.activation(e_sb[:], st_ps[:],
                                         mybir.ActivationFunctionType.Exp)
                    if causal and b >= d0:
                        nc.vector.tensor_mul(e_sb[:], e_sb[:],
                                             tri_sb[:, b - d0, 0:cw])
                    nc.tensor.matmul(ot[:], v_sb[:, b, :], e_sb[:],
                                     start=(b == 0), stop=(b == nb - 1))
                    with nc.allow_low_precision(reason="softmax denom bf16"):
                        if b == 0:
                            nc.vector.tensor_copy(esum[:], e_sb[:])
                        else:
                            nc.vector.tensor_add(esum[:], esum[:], e_sb[:])
                pending.append((key, c0, cw, nb, d0, h, ot, esum))
                if len(pending) > 2:
                    emit_tail(pending.pop(0))

            for u in pending:
                emit_tail(u)

    nc.compile()
    _BUILD_CACHE[key] = nc
    return nc


def _prep_inputs(hidden_states, attention_mask, cos, sin, Wq, Wk, Wv, Wo):
    X = np.asarray(hidden_states, dtype=np.float32).reshape(S, E)
    # [N_PC, 128, NE, PC]: exact SBUF tile layout per chunk -> long DMA runs
    XT4 = np.ascontiguousarray(
        X.reshape(N_PC, PC, NE, 128).transpose(0, 3, 2, 1)).astype(BF16)

    m = np.asarray(attention_mask, dtype=np.float32).reshape(S, S)
    il, ju = np.tril_indices(S), np.triu_indices(S, 1)
    causal = bool(np.all(m[il] == 0.0) and np.all(m[ju] <= -1e8))
    dense = bool(np.all(m == 0.0))
    if not (causal or dense):
        raise NotImplementedError("only causal or all-zero masks supported")

    scale = 1.0 / math.sqrt(D)
    cosT = np.ascontiguousarray(np.asarray(cos, np.float32).reshape(S, D).T)
    sinT = np.ascontiguousarray(np.asarray(sin, np.float32).reshape(S, D).T)
    sin_mod = sinT.copy()
    sin_mod[0:64] *= -1.0
    rope_t = np.stack([cosT * scale, sin_mod * scale, cosT, sin_mod]) \
        .astype(BF16)

    p = np.arange(128)[:, None]
    f = np.arange(AC)[None, :]
    tri = np.stack([(128 * jj + p <= f) for jj in range(4)]).astype(BF16)

    Wq = np.asarray(Wq, np.float32)
    Wk = np.asarray(Wk, np.float32)
    Wv = np.asarray(Wv, np.float32)
    Wo = np.asarray(Wo, np.float32)

    def wtile(Wshard):
        # [out, E] -> SBUF layout [128, NE, out]
        return np.ascontiguousarray(
            Wshard.T.reshape(NE, 128, Wshard.shape[0]).transpose(1, 0, 2)
        ).astype(BF16)

    in_maps = []
    for c in range(N_CORES):
        in_maps.append({
            "xt": XT4,
            "wqt": wtile(Wq[c * OSH:(c + 1) * OSH, :]),
            "wkt": wtile(Wk[c * D:(c + 1) * D, :]),
            "wvt": wtile(Wv[c * D:(c + 1) * D, :]),
            "wot": np.ascontiguousarray(
                Wo[:, c * OSH:(c + 1) * OSH].T.reshape(G, 128, E)
                .transpose(1, 0, 2)).astype(BF16),
            "rope": rope_t,
            "tri": tri,
        })
    return in_maps, causal


def kernel(hidden_states, attention_mask, cos, sin, Wq, Wk, Wv, Wo,
           _trace=False, _tmpdir=None):
    in_maps, causal = _prep_inputs(hidden_states, attention_mask, cos, sin,
                                   Wq, Wk, Wv, Wo)
    nc = build_bass(causal)
    res = run_bass_kernel_spmd(nc, in_maps, core_ids=list(range(N_CORES)),
                               trace=_trace, tmpdir=_tmpdir)
    out = res.results[0]["out"].astype(np.float32)
    for c in range(1, N_CORES):
        out = out + res.results[c]["out"]
    kernel._last_result = res
    return out.reshape(1, S, E).astype(np.float32)



# revision 5
# speedup vs baseline: 1.0707x; 1.0707x over previous
"""Trainium2 Bass kernel for Mllama-style GQA self-attention (B=1, S=2048,
H=32 q-heads, KVH=8 kv-heads, D=128), tensor-parallel over heads across 8
NeuronCores.

Sharding: core c owns kv-head c and q-heads 4c..4c+3 (Wq/Wk/Wv column shards),
computes its heads' attention output in transposed [feature, seq] layout, then
computes the full-width partial output projection rows (Wo row shard, own 512
features contracted); the host sums the 8 cores' fp32 partials.

All matmuls run in fp16 (fp32 PSUM accumulation) — same PE rate as bf16 but
2 extra mantissa bits. Softmax skips the max subtraction (scores are O(10)
so exp is safe; probs carry a 2^-4 exp bias that cancels in the normalize).
Projection chunks (512 queries) and attention chunks (256 queries, exact
block-causal) are emitted interleaved so the tensor queue always has
independent work; the softmax normalize tail is split into two stages
(den-matmul + fast-reciprocal immediately; broadcast + scale two units later)
so the tensor engine never waits on the vector chain.
"""
import math
import numpy as np
import ml_dtypes

import concourse.bass as bass
import concourse.bacc as bacc
import concourse.mybir as mybir
import concourse.tile as tile
from concourse.bass_utils import run_bass_kernel_spmd

F16 = np.float16
S, E, H, KVH, D = 2048, 4096, 32, 8, 128
N_CORES = 8
G = H // KVH                      # q heads per core (4)
OSH = G * D                       # per-core q/attn feature count (512)
PC = 512                          # projection seq chunk
N_PC = S // PC                    # 4
AC = 256                          # attention query chunk
N_AC = S // AC                    # 8
NE = E // 128                     # 32 contraction tiles
N_ST = S // 128                   # 16 seq tiles
LAG = 2                           # softmax-tail stage-B pipeline depth
EXP_BIAS = -2.772588722239781     # -4*ln2: probs scaled 2^-4, cancels in norm

_BUILD_CACHE = {}


def build_bass(causal: bool):
    key = causal
    if key in _BUILD_CACHE:
        return _BUILD_CACHE[key]
    dt = mybir.dt
    nc = bacc.Bacc("TRN2", target_bir_lowering=False, debug=False,
                   enable_asserts=False, num_devices=N_CORES)

    XT4 = nc.dram_tensor("xt", [N_PC, 128, NE, PC], dt.float16, kind="ExternalInput").ap()
    WQT = nc.dram_tensor("wqt", [128, NE, OSH], dt.float16, kind="ExternalInput").ap()
    WKT = nc.dram_tensor("wkt", [128, NE, D], dt.float16, kind="ExternalInput").ap()
    WVT = nc.dram_tensor("wvt", [128, NE, D], dt.float16, kind="ExternalInput").ap()
    WOT = nc.dram_tensor("wot", [128, G, E], dt.float16, kind="ExternalInput").ap()
    ROPE = nc.dram_tensor("rope", [4, D, S], dt.float16, kind="ExternalInput").ap()
    TRI = nc.dram_tensor("tri", [2, 128, AC], dt.bfloat16, kind="ExternalInput").ap()
    OUT = nc.dram_tensor("out", [S, E], dt.float32, kind="ExternalOutput").ap()

    with tile.TileContext(nc) as tc:
        with (
            tc.tile_pool(name="wpool", bufs=1) as wpool,
            tc.tile_pool(name="qkv", bufs=1) as qkvpool,
            tc.tile_pool(name="consts", bufs=1) as cpool,
            tc.tile_pool(name="xs", bufs=2) as xspool,
            tc.tile_pool(name="cs", bufs=2) as cspool,
            tc.tile_pool(name="rtmp", bufs=1) as rtmppool,
            tc.tile_pool(name="epool", bufs=3) as epool,
            tc.tile_pool(name="small", bufs=4) as smallpool,
            tc.tile_pool(name="attn", bufs=9) as attnpool,
            tc.tile_pool(name="outs", bufs=4) as outpool,
            tc.tile_pool(name="ps_qkv", bufs=2, space="PSUM") as ps_qkv,
            tc.tile_pool(name="ps_s", bufs=3, space="PSUM") as ps_s,
            tc.tile_pool(name="ps_ot", bufs=3, space="PSUM") as ps_ot,
        ):
            # --- resident weights, [128, NE, width]. Priority order: the
            # first chunk's activations and Wk jump ahead of the big weight
            # burst so the PE starts fast.
            wq_sb = wpool.tile([128, NE, OSH], dt.float16)
            wk_sb = wpool.tile([128, NE, D], dt.float16)
            wv_sb = wpool.tile([128, NE, D], dt.float16)
            wo_sb = wpool.tile([128, G, E], dt.float16)

            xs0 = xspool.tile([128, NE, PC], dt.float16, tag="xs")
            cs0 = cspool.tile([128, 4, PC], dt.float16, tag="cs")
            for q in range(8):
                nc.sync.dma_start(wk_sb[:, q * 4:(q + 1) * 4, :],
                                  WKT[:, q * 4:(q + 1) * 4, :])
                nc.sync.dma_start(xs0[:, q * 4:(q + 1) * 4, :],
                                  XT4[0, :, q * 4:(q + 1) * 4, :])
            for q in range(8):
                nc.sync.dma_start(wv_sb[:, q * 4:(q + 1) * 4, :],
                                  WVT[:, q * 4:(q + 1) * 4, :])
            nc.sync.dma_start(cs0[:], ROPE[:, :, 0:PC].rearrange("j p s -> p j s"))
            for q in range(8):
                nc.sync.dma_start(wq_sb[:, q * 4:(q + 1) * 4, :],
                                  WQT[:, q * 4:(q + 1) * 4, :])
            for q in range(4):
                nc.sync.dma_start(wo_sb[:, :, q * 1024:(q + 1) * 1024],
                                  WOT[:, :, q * 1024:(q + 1) * 1024])

            tri_sb = cpool.tile([128, 2, AC], dt.bfloat16)
            nc.sync.dma_start(tri_sb[:], TRI.rearrange("j p f -> p j f"))
            ones_col = cpool.tile([128, 1], dt.bfloat16)
            nc.vector.memset(ones_col[:], 1.0)
            ones_row = cpool.tile([1, 128], dt.float32)
            nc.vector.memset(ones_row[:], 1.0)
            ebias = cpool.tile([128, 1], dt.float32)
            nc.vector.memset(ebias[:], EXP_BIAS)

            # --- persistent activations
            qT_sb = qkvpool.tile([128, G, S], dt.float16)     # per-head [d, s]
            kT_sb = qkvpool.tile([128, S], dt.float16)        # [d, s]
            v_sb = qkvpool.tile([128, N_ST, D], dt.bfloat16)   # per s-tile [t, d]

            # --- softmax-tail pipeline state
            pendingB = []
            attnTs = {c: {} for c in range(N_AC)}
            remaining = {c: G for c in range(N_AC)}

            def emit_oproj(c):
                c0 = c * AC
                ats = attnTs[c]
                for t in range(AC // 128):
                    for pc8 in range(8):
                        po = ps_qkv.tile([128, OSH], dt.float32, tag="pq",
                                         name="po")
                        for hh in range(G):
                            nc.tensor.matmul(
                                po[:], ats[hh][:, t * 128:(t + 1) * 128],
                                wo_sb[:, hh, pc8 * OSH:(pc8 + 1) * OSH],
                                start=(hh == 0), stop=(hh == G - 1))
                        o_sb = outpool.tile([128, OSH], dt.float32, tag="o")
                        if pc8 % 2 == 0:
                            nc.vector.tensor_copy(o_sb[:], po[:])
                        else:
                            nc.scalar.copy(o_sb[:], po[:])
                        nc.sync.dma_start(
                            OUT[c0 + t * 128: c0 + (t + 1) * 128,
                                pc8 * OSH:(pc8 + 1) * OSH], o_sb[:])

            def emit_stageB(u):
                c, h, ot, recip = u
                bc_ps = ps_s.tile([128, AC], dt.float32, tag="st", name="bc")
                nc.tensor.matmul(bc_ps[:], ones_row[:], recip[:],
                                 start=True, stop=True)
                bc_sb = smallpool.tile([128, AC], dt.float16, tag="bc_sb")
                nc.scalar.copy(bc_sb[:], bc_ps[:])
                attnT = attnpool.tile([128, AC], dt.float16, tag="attnT")
                nc.vector.tensor_mul(attnT[:], ot[:], bc_sb[:])
                attnTs[c][h] = attnT
                remaining[c] -= 1
                if remaining[c] == 0:
                    emit_oproj(c)

            def emit_unit(c, h):
                c0 = c * AC
                nb = 2 * (c + 1) if causal else N_ST
                d0 = 2 * c if causal else N_ST
                ot = ps_ot.tile([128, AC], dt.float32, tag="ot")
                esum = epool.tile([128, AC], dt.bfloat16, tag="esum")
                for b in range(nb):
                    st_ps = ps_s.tile([128, AC], dt.float32, tag="st")
                    nc.tensor.matmul(st_ps[:], kT_sb[:, b * 128:(b + 1) * 128],
                                     qT_sb[:, h, c0:c0 + AC],
                                     start=True, stop=True)
                    e_sb = epool.tile([128, AC], dt.bfloat16, tag="e")
                    nc.scalar.activation(e_sb[:], st_ps[:],
                                         mybir.ActivationFunctionType.Exp,
                                         bias=ebias[:])
                    if causal and b >= d0:
                        nc.vector.tensor_mul(e_sb[:], e_sb[:],
                                             tri_sb[:, b - d0, :])
                    nc.tensor.matmul(ot[:], v_sb[:, b, :], e_sb[:],
                                     start=(b == 0), stop=(b == nb - 1))
                    with nc.allow_low_precision(reason="softmax denom f16"):
                        if b == 0:
                            nc.vector.tensor_copy(esum[:], e_sb[:])
                        else:
                            nc.vector.tensor_add(esum[:], esum[:], e_sb[:])
                # stage A: denominator + fast reciprocal (off the PE path)
                den = ps_s.tile([1, AC], dt.float32, tag="st", name="den")
                nc.tensor.matmul(den[:], ones_col[:], esum[:],
                                 start=True, stop=True)
                recip = smallpool.tile([1, AC], dt.float32, tag="recip")
                nc.vector.reciprocal_approx_fast(recip[:], den[:])
                pendingB.append((c, h, ot, recip))
                while len(pendingB) > LAG:
                    emit_stageB(pendingB.pop(0))

            # ===== merged pipeline: QKV proj chunk, then its attention =====
            for sc in range(N_PC):
                s0 = sc * PC
                if sc == 0:
                    xs, cs = xs0, cs0
                else:
                    xs = xspool.tile([128, NE, PC], dt.float16, tag="xs")
                    for q in range(4):
                        nc.sync.dma_start(xs[:, q * 8:(q + 1) * 8, :],
                                          XT4[sc, :, q * 8:(q + 1) * 8, :])
                    cs = cspool.tile([128, 4, PC], dt.float16, tag="cs")
                    nc.sync.dma_start(cs[:], ROPE[:, :, s0:s0 + PC]
                                      .rearrange("j p s -> p j s"))
                cosq_t, sinq_t = cs[:, 0, :], cs[:, 1, :]
                cosk_t, sink_t = cs[:, 2, :], cs[:, 3, :]

                # k head first, then v, then q heads (k/v weights land first)
                for hh in [G, -1] + list(range(G)):
                    if hh == -1:
                        for u in range(PC // 128):
                            st = (s0 // 128) + u
                            pv = ps_qkv.tile([128, D], dt.float32, tag="pq")
                            for e in range(NE):
                                nc.tensor.matmul(pv[:],
                                                 xs[:, e, u * 128:(u + 1) * 128],
                                                 wv_sb[:, e, :],
                                                 start=(e == 0), stop=(e == NE - 1))
                            nc.vector.tensor_copy(v_sb[:, st, :], pv[:])
                        continue
                    is_k = hh == G
                    pq = ps_qkv.tile([128, PC], dt.float32, tag="pq")
                    for e in range(NE):
                        lhsT = (wk_sb[:, e, :] if is_k
                                else wq_sb[:, e, hh * D:(hh + 1) * D])
                        nc.tensor.matmul(pq[:], lhsT, xs[:, e, :],
                                         start=(e == 0), stop=(e == NE - 1))
                    cos_t, sin_t = (cosk_t, sink_t) if is_k else (cosq_t, sinq_t)
                    dest = kT_sb[:, s0:s0 + PC] if is_k \
                        else qT_sb[:, hh, s0:s0 + PC]
                    rt = rtmppool.tile([128, 2, PC], dt.float32, tag="rt")
                    t1, t2 = rt[:, 0, :], rt[:, 1, :]
                    # low half: q'= q_lo*cos_lo + q_hi*sin_mod_lo
                    nc.vector.tensor_mul(t1[0:64, :], pq[0:64, :], cos_t[0:64, :])
                    nc.vector.tensor_mul(t2[0:64, :], pq[64:128, :], sin_t[0:64, :])
                    nc.vector.tensor_add(dest[0:64, :], t1[0:64, :], t2[0:64, :])
                    # high half: q'= q_hi*cos_hi + q_lo*sin_mod_hi
                    nc.vector.tensor_mul(t1[64:128, :], pq[64:128, :], cos_t[64:128, :])
                    nc.vector.tensor_mul(t2[64:128, :], pq[0:64, :], sin_t[64:128, :])
                    nc.vector.tensor_add(dest[64:128, :], t1[64:128, :], t2[64:128, :])

                # attention chunks covered by projections so far
                for ac_ in (2 * sc, 2 * sc + 1):
                    for h in range(G):
                        emit_unit(ac_, h)

            while pendingB:
                emit_stageB(pendingB.pop(0))

    nc.compile()
    _BUILD_CACHE[key] = nc
    return nc


def _prep_inputs(hidden_states, attention_mask, cos, sin, Wq, Wk, Wv, Wo):
    X = np.asarray(hidden_states, dtype=np.float32).reshape(S, E)
    # [N_PC, 128, NE, PC]: exact SBUF tile layout per chunk -> long DMA runs
    XT4 = np.ascontiguousarray(
        X.reshape(N_PC, PC, NE, 128).transpose(0, 3, 2, 1)).astype(F16)

    m = np.asarray(attention_mask, dtype=np.float32).reshape(S, S)
    il, ju = np.tril_indices(S), np.triu_indices(S, 1)
    causal = bool(np.all(m[il] == 0.0) and np.all(m[ju] <= -1e8))
    dense = bool(np.all(m == 0.0))
    if not (causal or dense):
        raise NotImplementedError("only causal or all-zero masks supported")

    scale = 1.0 / math.sqrt(D)
    cosT = np.ascontiguousarray(np.asarray(cos, np.float32).reshape(S, D).T)
    sinT = np.ascontiguousarray(np.asarray(sin, np.float32).reshape(S, D).T)
    sin_mod = sinT.copy()
    sin_mod[0:64] *= -1.0
    rope_t = np.stack([cosT * scale, sin_mod * scale, cosT, sin_mod]) \
        .astype(F16)

    p = np.arange(128)[:, None]
    f = np.arange(AC)[None, :]
    tri = np.stack([(128 * jj + p <= f) for jj in range(2)]).astype(ml_dtypes.bfloat16)

    Wq = np.asarray(Wq, np.float32)
    Wk = np.asarray(Wk, np.float32)
    Wv = np.asarray(Wv, np.float32)
    Wo = np.asarray(Wo, np.float32)

    def wtile(Wshard):
        # [out, E] -> SBUF layout [128, NE, out]
        return np.ascontiguousarray(
            Wshard.T.reshape(NE, 128, Wshard.shape[0]).transpose(1, 0, 2)
        ).astype(F16)

    in_maps = []
    for c in range(N_CORES):
        in_maps.append({
            "xt": XT4,
            "wqt": wtile(Wq[c * OSH:(c + 1) * OSH, :]),
            "wkt": wtile(Wk[c * D:(c + 1) * D, :]),
            "wvt": wtile(Wv[c * D:(c + 1) * D, :]),
            "wot": np.ascontiguousarray(
                Wo[:, c * OSH:(c + 1) * OSH].T.reshape(G, 128, E)
                .transpose(1, 0, 2)).astype(F16),
            "rope": rope_t,
            "tri": tri,
        })
    return in_maps, causal


def kernel(hidden_states, attention_mask, cos, sin, Wq, Wk, Wv, Wo,
           _trace=False, _tmpdir=None):
    in_maps, causal = _prep_inputs(hidden_states, attention_mask, cos, sin,
                                   Wq, Wk, Wv, Wo)
    nc = build_bass(causal)
    res = run_bass_kernel_spmd(nc, in_maps, core_ids=list(range(N_CORES)),
                               trace=_trace, tmpdir=_tmpdir)
    out = res.results[0]["out"].astype(np.float32)
    for c in range(1, N_CORES):
        out = out + res.results[c]["out"]
    kernel._last_result = res
    return out.reshape(1, S, E).astype(np.float32)


# revision 8
# speedup vs baseline: 1.3106x; 1.2240x over previous
"""Trainium2 Bass kernel for Mllama-style GQA self-attention (B=1, S=2048,
H=32 q-heads, KVH=8 kv-heads, D=128), tensor-parallel over heads across 8
NeuronCores.

Sharding: core c owns kv-head c and q-heads 4c..4c+3 (Wq/Wk/Wv column shards),
computes its heads' attention output in transposed [feature, seq] layout, then
computes the full-width partial output projection rows (Wo row shard, own 512
features contracted); the host sums the 8 cores' fp32 partials.

Matmuls run in fp16 (fp32 PSUM accumulation); the exp/probs path is bf16 for
range (scores reach ~17, exp overflows fp16). Attention runs on 256-query
chunks (exact block-causal) with the two heads of each GQA pair fused into
single 512-wide score/PV matmuls (one PSUM accumulation group per pair).
A quantum scheduler interleaves o_proj groups and the next projection chunk's
matmuls between each round's score and PV blocks, hiding the scalar-engine
exp latency so the in-order tensor queue never waits. The softmax normalize
uses a ones-matmul denominator + fast DVE reciprocal + ones-broadcast matmul,
pipelined across chunk boundaries.
"""
import math
from collections import deque
import numpy as np
import ml_dtypes

import concourse.bass as bass
import concourse.bacc as bacc
import concourse.mybir as mybir
import concourse.tile as tile
from concourse.bass_utils import run_bass_kernel_spmd

F16 = np.float16
BF16 = ml_dtypes.bfloat16
S, E, H, KVH, D = 2048, 4096, 32, 8, 128
N_CORES = 8
G = H // KVH                      # q heads per core (4)
NP = G // 2                       # head pairs per core (2)
OSH = G * D                       # per-core q/attn feature count (512)
PC = 512                          # projection seq chunk
N_PC = S // PC                    # 4
AC = 256                          # attention query chunk
AC2 = 2 * AC                      # paired width (512)
N_AC = S // AC                    # 8
NE = E // 128                     # 32 contraction tiles
N_ST = S // 128                   # 16 seq tiles
EXP_BIAS = -2.772588722239781     # -4*ln2: probs scaled 2^-4, cancels in norm

_BUILD_CACHE = {}


def build_bass(causal: bool):
    key = causal
    if key in _BUILD_CACHE:
        return _BUILD_CACHE[key]
    dt = mybir.dt
    nc = bacc.Bacc("TRN2", target_bir_lowering=False, debug=False,
                   enable_asserts=False, num_devices=N_CORES)

    XT4 = nc.dram_tensor("xt", [N_PC, 128, NE, PC], dt.float16, kind="ExternalInput").ap()
    WQT = nc.dram_tensor("wqt", [128, NE, OSH], dt.float16, kind="ExternalInput").ap()
    WKT = nc.dram_tensor("wkt", [128, NE, D], dt.float16, kind="ExternalInput").ap()
    WVT = nc.dram_tensor("wvt", [128, NE, D], dt.float16, kind="ExternalInput").ap()
    WOT = nc.dram_tensor("wot", [128, G, E], dt.float16, kind="ExternalInput").ap()
    ROPE = nc.dram_tensor("rope", [4, D, S], dt.float16, kind="ExternalInput").ap()
    TRI = nc.dram_tensor("tri", [2, 128, AC2], dt.bfloat16, kind="ExternalInput").ap()
    OUT = nc.dram_tensor("out", [S, E], dt.float32, kind="ExternalOutput").ap()

    with tile.TileContext(nc) as tc:
        with (
            tc.tile_pool(name="wpool", bufs=1) as wpool,
            tc.tile_pool(name="qkv", bufs=1) as qkvpool,
            tc.tile_pool(name="consts", bufs=1) as cpool,
            tc.tile_pool(name="xs", bufs=2) as xspool,
            tc.tile_pool(name="cs", bufs=2) as cspool,
            tc.tile_pool(name="rtmp", bufs=1) as rtmppool,
            tc.tile_pool(name="epool", bufs=3) as epool,
            tc.tile_pool(name="small", bufs=2) as smallpool,
            tc.tile_pool(name="attn", bufs=5) as attnpool,
            tc.tile_pool(name="outs", bufs=3) as outpool,
            tc.tile_pool(name="ps_qkv", bufs=2, space="PSUM") as ps_qkv,
            tc.tile_pool(name="ps_s", bufs=3, space="PSUM") as ps_s,
            tc.tile_pool(name="ps_ot", bufs=3, space="PSUM") as ps_ot,
        ):
            # --- resident weights, [128, NE, width]. Priority order: the
            # first chunk's activations and Wk jump ahead of the big weight
            # burst so the PE starts fast.
            wq_sb = wpool.tile([128, NE, OSH], dt.float16)
            wk_sb = wpool.tile([128, NE, D], dt.float16)
            wv_sb = wpool.tile([128, NE, D], dt.float16)
            wo_sb = wpool.tile([128, G, E], dt.float16)

            xs0 = xspool.tile([128, NE, PC], dt.float16, tag="xs")
            cs0 = cspool.tile([128, 4, PC], dt.float16, tag="cs")
            for q in range(8):
                nc.sync.dma_start(wk_sb[:, q * 4:(q + 1) * 4, :],
                                  WKT[:, q * 4:(q + 1) * 4, :])
                nc.sync.dma_start(xs0[:, q * 4:(q + 1) * 4, :],
                                  XT4[0, :, q * 4:(q + 1) * 4, :])
            for q in range(8):
                nc.sync.dma_start(wv_sb[:, q * 4:(q + 1) * 4, :],
                                  WVT[:, q * 4:(q + 1) * 4, :])
            nc.sync.dma_start(cs0[:], ROPE[:, :, 0:PC].rearrange("j p s -> p j s"))
            for q in range(8):
                nc.sync.dma_start(wq_sb[:, q * 4:(q + 1) * 4, :],
                                  WQT[:, q * 4:(q + 1) * 4, :])
            for q in range(4):
                nc.sync.dma_start(wo_sb[:, :, q * 1024:(q + 1) * 1024],
                                  WOT[:, :, q * 1024:(q + 1) * 1024])

            tri_sb = cpool.tile([128, 2, AC2], dt.bfloat16)
            nc.sync.dma_start(tri_sb[:], TRI.rearrange("j p f -> p j f"))
            ones_col = cpool.tile([128, 1], dt.bfloat16)
            nc.vector.memset(ones_col[:], 1.0)
            ones_row = cpool.tile([1, 128], dt.float32)
            nc.vector.memset(ones_row[:], 1.0)
            ebias = cpool.tile([128, 1], dt.float32)
            nc.vector.memset(ebias[:], EXP_BIAS)

            # --- persistent activations
            # qT: [d, chunk, head, within-chunk] so a head PAIR's queries for
            # one attention chunk are one contiguous 512-wide rhs.
            qT_sb = qkvpool.tile([128, N_AC, G, AC], dt.float16)
            kT_sb = qkvpool.tile([128, S], dt.float16)        # [d, s]
            v_sb = qkvpool.tile([128, N_ST, D], dt.bfloat16)  # per s-tile [t, d]

            # --- quantum scheduler state
            proj_q = deque()
            oproj_q = deque()

            def pump(n):
                for _ in range(n):
                    if proj_q:
                        proj_q.popleft()()
                    elif oproj_q:
                        oproj_q.popleft()()

            def flush(q):
                while q:
                    q.popleft()()

            # ---------- projection chunk -> quanta ----------
            def push_proj(sc):
                s0 = sc * PC
                if sc == 0:
                    xs, cs = xs0, cs0
                else:
                    xs = xspool.tile([128, NE, PC], dt.float16, tag="xs",
                                     name="xs")
                    for q in range(4):
                        nc.sync.dma_start(xs[:, q * 8:(q + 1) * 8, :],
                                          XT4[sc, :, q * 8:(q + 1) * 8, :])
                    cs = cspool.tile([128, 4, PC], dt.float16, tag="cs",
                                     name="cs")
                    nc.sync.dma_start(cs[:], ROPE[:, :, s0:s0 + PC]
                                      .rearrange("j p s -> p j s"))
                st8 = {}

                def rope_evac(hh, pq):
                    is_k = hh == G
                    cos_t = cs[:, 2, :] if is_k else cs[:, 0, :]
                    sin_t = cs[:, 3, :] if is_k else cs[:, 1, :]
                    rt = rtmppool.tile([128, 2, PC], dt.float32, tag="rt",
                                       name="rt")
                    t1, t2 = rt[:, 0, :], rt[:, 1, :]
                    nc.vector.tensor_mul(t1[0:64, :], pq[0:64, :], cos_t[0:64, :])
                    nc.vector.tensor_mul(t2[0:64, :], pq[64:128, :], sin_t[0:64, :])
                    nc.vector.tensor_mul(t1[64:128, :], pq[64:128, :], cos_t[64:128, :])
                    nc.vector.tensor_mul(t2[64:128, :], pq[0:64, :], sin_t[64:128, :])
                    if is_k:
                        nc.vector.tensor_add(kT_sb[:, s0:s0 + PC], t1[:], t2[:])
                    else:
                        for i in range(2):
                            nc.vector.tensor_add(
                                qT_sb[:, 2 * sc + i, hh, :],
                                t1[:, i * AC:(i + 1) * AC],
                                t2[:, i * AC:(i + 1) * AC])

                def q_quant(hh, part):
                    def th():
                        if part == 0:
                            st8[hh] = ps_qkv.tile([128, PC], dt.float32,
                                                  tag="pq", name="pq")
                        pq = st8[hh]
                        is_k = hh == G
                        for e in range(part * 4, part * 4 + 4):
                            lhsT = (wk_sb[:, e, :] if is_k
                                    else wq_sb[:, e, hh * D:(hh + 1) * D])
                            nc.tensor.matmul(pq[:], lhsT, xs[:, e, :],
                                             start=(e == 0), stop=(e == NE - 1))
                        if part == 7:
                            rope_evac(hh, pq)
                    return th

                def v_quant(u, part):
                    def th():
                        if part == 0:
                            st8[('v', u)] = ps_qkv.tile([128, D], dt.float32,
                                                        tag="pq", name="pv")
                        pv = st8[('v', u)]
                        for e in range(part * 8, part * 8 + 8):
                            nc.tensor.matmul(pv[:],
                                             xs[:, e, u * 128:(u + 1) * 128],
                                             wv_sb[:, e, :],
                                             start=(e == 0), stop=(e == NE - 1))
                        if part == 3:
                            nc.vector.tensor_copy(v_sb[:, (s0 // 128) + u, :],
                                                  pv[:])
                    return th

                for part in range(8):
                    proj_q.append(q_quant(G, part))       # K first
                for u in range(PC // 128):
                    for part in range(4):
                        proj_q.append(v_quant(u, part))   # then V
                for hh in range(G):
                    for part in range(8):
                        proj_q.append(q_quant(hh, part))  # then Q heads

            # ---------- attention ----------
            attnTs = {c: {} for c in range(N_AC)}
            pendB = []          # (c, p, ot2, recip) awaiting stage B

            def push_oproj(c):
                ats = attnTs[c]
                c0 = c * AC

                def grp(t, pc8):
                    def th():
                        po = ps_qkv.tile([128, OSH], dt.float32, tag="pq",
                                         name="po")
                        for hh in range(G):
                            nc.tensor.matmul(
                                po[:],
                                ats[hh // 2][:, (hh % 2) * AC + t * 128:
                                             (hh % 2) * AC + (t + 1) * 128],
                                wo_sb[:, hh, pc8 * OSH:(pc8 + 1) * OSH],
                                start=(hh == 0), stop=(hh == G - 1))
                        o_sb = outpool.tile([128, OSH], dt.float32, tag="o")
                        if pc8 % 2 == 0:
                            nc.vector.tensor_copy(o_sb[:], po[:])
                        else:
                            nc.scalar.copy(o_sb[:], po[:])
                        nc.sync.dma_start(
                            OUT[c0 + t * 128: c0 + (t + 1) * 128,
                                pc8 * OSH:(pc8 + 1) * OSH], o_sb[:])
                    return th

                for t in range(AC // 128):
                    for pc8 in range(8):
                        oproj_q.append(grp(t, pc8))

            def stageB_flush():
                flush(oproj_q)
                pump(2)
                done = []
                for (c, p, ot2, recip) in pendB:
                    bc_ps = ps_s.tile([128, AC2], dt.float32, tag="st",
                                      name="bc")
                    nc.tensor.matmul(bc_ps[:], ones_row[:], recip[:],
                                     start=True, stop=True)
                    bc_sb = smallpool.tile([128, AC2], dt.float16, tag="bc_sb")
                    nc.scalar.copy(bc_sb[:], bc_ps[:])
                    attnT2 = attnpool.tile([128, AC2], dt.float16, tag="attnT")
                    nc.vector.tensor_mul(attnT2[:], ot2[:], bc_sb[:])
                    attnTs[c][p] = attnT2
                    done.append(c)
                pendB.clear()
                for c in sorted(set(done)):
                    push_oproj(c)

            def att_chunk(c):
                nb = 2 * (c + 1) if causal else N_ST
                d0 = 2 * c if causal else N_ST
                ots = [ps_ot.tile([128, AC2], dt.float32, tag="ot", name="ot")
                       for _ in range(NP)]
                esums = [epool.tile([128, AC2], dt.bfloat16, tag="esum",
                                    name="esum", bufs=2) for _ in range(NP)]
                for b in range(nb):
                    diag = causal and b >= d0
                    e2s = []
                    for p in range(NP):
                        st2 = ps_s.tile([128, AC2], dt.float32, tag="st",
                                        name="st")
                        nc.tensor.matmul(st2[:],
                                         kT_sb[:, b * 128:(b + 1) * 128],
                                         qT_sb[:, c, 2 * p:2 * p + 2, :],
                                         start=True, stop=True)
                        e2 = epool.tile([128, AC2], dt.bfloat16, tag="e",
                                        name="e2")
                        nc.scalar.activation(e2[:], st2[:],
                                             mybir.ActivationFunctionType.Exp,
                                             bias=ebias[:])
                        if diag:
                            nc.vector.tensor_mul(e2[:], e2[:],
                                                 tri_sb[:, b - d0, :])
                        e2s.append(e2)
                    pump(2 if diag else 1)
                    for p in range(NP):
                        nc.tensor.matmul(ots[p][:], v_sb[:, b, :], e2s[p][:],
                                         start=(b == 0), stop=(b == nb - 1))
                        with nc.allow_low_precision(reason="softmax denom"):
                            if b == 0:
                                nc.vector.tensor_copy(esums[p][:], e2s[p][:])
                            else:
                                nc.vector.tensor_add(esums[p][:], esums[p][:],
                                                     e2s[p][:])
                # stage A: denominator + fast reciprocal (off the PE path)
                for p in range(NP):
                    den = ps_s.tile([1, AC2], dt.float32, tag="st", name="den")
                    nc.tensor.matmul(den[:], ones_col[:], esums[p][:],
                                     start=True, stop=True)
                    recip = smallpool.tile([1, AC2], dt.float32, tag="recip")
                    nc.vector.reciprocal_approx_fast(recip[:], den[:])
                    pendB.append((c, p, ots[p], recip))

            # ===================== driver =====================
            push_proj(0)
            flush(proj_q)
            for c in range(N_AC):
                if c % 2 == 0 and c // 2 + 1 < N_PC:
                    push_proj(c // 2 + 1)
                stageB_flush()
                att_chunk(c)
                if c % 2 == 1:
                    flush(proj_q)
            stageB_flush()
            flush(oproj_q)

    nc.compile()
    _BUILD_CACHE[key] = nc
    return nc


def _prep_inputs(hidden_states, attention_mask, cos, sin, Wq, Wk, Wv, Wo):
    X = np.asarray(hidden_states, dtype=np.float32).reshape(S, E)
    # [N_PC, 128, NE, PC]: exact SBUF tile layout per chunk -> long DMA runs
    XT4 = np.ascontiguousarray(
        X.reshape(N_PC, PC, NE, 128).transpose(0, 3, 2, 1)).astype(F16)

    m = np.asarray(attention_mask, dtype=np.float32).reshape(S, S)
    il, ju = np.tril_indices(S), np.triu_indices(S, 1)
    causal = bool(np.all(m[il] == 0.0) and np.all(m[ju] <= -1e8))
    dense = bool(np.all(m == 0.0))
    if not (causal or dense):
        raise NotImplementedError("only causal or all-zero masks supported")

    scale = 1.0 / math.sqrt(D)
    cosT = np.ascontiguousarray(np.asarray(cos, np.float32).reshape(S, D).T)
    sinT = np.ascontiguousarray(np.asarray(sin, np.float32).reshape(S, D).T)
    sin_mod = sinT.copy()
    sin_mod[0:64] *= -1.0
    rope_t = np.stack([cosT * scale, sin_mod * scale, cosT, sin_mod]) \
        .astype(F16)

    p = np.arange(128)[:, None]
    f = np.arange(AC)[None, :]
    tri1 = np.stack([(128 * jj + p <= f) for jj in range(2)])
    tri = np.concatenate([tri1, tri1], axis=2).astype(BF16)  # [2,128,2*AC]

    Wq = np.asarray(Wq, np.float32)
    Wk = np.asarray(Wk, np.float32)
    Wv = np.asarray(Wv, np.float32)
    Wo = np.asarray(Wo, np.float32)

    def wtile(Wshard):
        # [out, E] -> SBUF layout [128, NE, out]
        return np.ascontiguousarray(
            Wshard.T.reshape(NE, 128, Wshard.shape[0]).transpose(1, 0, 2)
        ).astype(F16)

    in_maps = []
    for c in range(N_CORES):
        in_maps.append({
            "xt": XT4,
            "wqt": wtile(Wq[c * OSH:(c + 1) * OSH, :]),
            "wkt": wtile(Wk[c * D:(c + 1) * D, :]),
            "wvt": wtile(Wv[c * D:(c + 1) * D, :]),
            "wot": np.ascontiguousarray(
                Wo[:, c * OSH:(c + 1) * OSH].T.reshape(G, 128, E)
                .transpose(1, 0, 2)).astype(F16),
            "rope": rope_t,
            "tri": tri,
        })
    return in_maps, causal


def kernel(hidden_states, attention_mask, cos, sin, Wq, Wk, Wv, Wo,
           _trace=False, _tmpdir=None):
    in_maps, causal = _prep_inputs(hidden_states, attention_mask, cos, sin,
                                   Wq, Wk, Wv, Wo)
    nc = build_bass(causal)
    res = run_bass_kernel_spmd(nc, in_maps, core_ids=list(range(N_CORES)),
                               trace=_trace, tmpdir=_tmpdir)
    out = res.results[0]["out"].astype(np.float32)
    for c in range(1, N_CORES):
        out = out + res.results[c]["out"]
    kernel._last_result = res
    return out.reshape(1, S, E).astype(np.float32)
